# revision 5
# baseline (speedup 1.0000x reference)
"""Trainium2 Bass kernel for nn_BezierButtress (Bernstein-basis permutation chains).

Math (per permutation chain p, over depth d = 0..31):
    S_mean <- (S_mean @ Wm_d) * B(x_{perm[p,d]})        (K=17 wide state)
    S_var  <- (S_var  @ Wv_d) * B(x_{perm[p,d]})^2
    outputs: f_mean[n] = sum_{p,k} S_mean, f_var[n] = sum_{p,k} S_var / post_prec[p]

Device strategy (data-parallel over N across 8 cores, 3072 rows each):
  * all matmuls in bf16 (PE streams 2x faster than fp32r); chain states,
    weights and multipliers are bf16 -- the rel-err gate is 2e-2, bf16
    chains land ~1e-3.
  * basis tables are built ONCE per (feature, k) per chunk instead of once
    per (chain, step): a gather matmul contracts a per-feature-group
    selection matrix against the resident log-table (hi/lo bf16 split in
    the contraction) giving logB = k*log(x_f) + (16-k)*log(1-x_f) for
    7 features x 17 k on 119 partitions; ACT exps it into SBUF slabs
    B (bias=log binom) and B^2 (scale=2, bias=2 log binom).  This cuts
    ACT exp work ~10x vs exp-ing per (d, g) tile.
  * per-step multipliers [7 chains x 17 k, 2, n] are DMA-gathered
    SBUF->SBUF from the slabs (7 partition-block copies per tile, each
    covering the B/B^2 pair; DMA engines are otherwise idle).  d=0
    gathers straight into the chain state (meanw0/varw0 prefactors are
    folded into the d=1 weights).
  * PSUM drain split in an [S,S,D] tile rotation: on S-tiles the DVE
    multiplies the mean chain out of PSUM while ACT copies the var chain
    to SBUF (bf16) and GPSIMD does the var multiply from SBUF; on D-tiles
    the DVE drains both.  This balances the 1-elem/cycle PSUM read ports
    (DVE+ACT) that bound this kernel.
"""

import os
import numpy as np
import ml_dtypes
from math import comb

import concourse.bass as bass
import concourse.mybir as mybir
import concourse.tile as tile
from concourse import bacc
from concourse import bass_utils

ORDER = 16
K = 17
D = 32
P = 20
N = 24576
NCORES = 8
NLOC = N // NCORES        # 3072
CPG = 7                   # chain slots per group
G = 3                     # groups (7, 7, 6 + 1 pad)
FPG = 7                   # features per basis slab
FG = 5                    # feature groups (7,7,7,7,4)
R = CPG * K               # 119 active partitions
RP = 128                  # padded partition count
CHUNK = 1024
HALF = 512
F32 = mybir.dt.float32
BF16 = mybir.dt.bfloat16
EXP = mybir.ActivationFunctionType.Exp
MULT = mybir.AluOpType.mult

# set by _host_tensors (the gather offsets depend on perm)
perm_host = None


def _flags():
    # BB_SMODE: tiles out of 3 that use the split drain (ACT evacuates the
    # var chain, GPSIMD multiplies it).  0 = all drains on DVE.
    smode = int(os.environ.get("BB_SMODE", "2"))
    return smode


def _bf16_split(x64):
    hi = x64.astype(ml_dtypes.bfloat16)
    lo = (x64 - hi.astype(np.float64)).astype(ml_dtypes.bfloat16)
    return hi, lo


def _host_tensors(Xnew, meanw0, meanw, varw0, varw, prior_sc, post_prec, perm):
    global perm_host
    perm_host = np.asarray(perm)
    Xnew = np.asarray(Xnew, np.float32)
    meanw0 = np.asarray(meanw0, np.float64)   # (P, 1, K)
    meanw = np.asarray(meanw, np.float64)     # (D-1, P, K, K)
    varw0 = np.asarray(varw0, np.float64)     # (P, 1, K)
    varw = np.asarray(varw, np.float64)       # (D-1, P, K, K)
    prior_sc = np.asarray(prior_sc, np.float64)  # (K, 1)
    post_prec = np.asarray(post_prec, np.float64)  # (P,)

    # --- per-core UV log tables (hi/lo bf16 split on the contraction) --
    x64 = np.clip(Xnew.astype(np.float64), 1e-30, None)
    u64 = np.log(x64)                                    # (N, D)
    v64 = np.log1p(-np.minimum(Xnew.astype(np.float64), 1.0 - 1e-15))
    uh, ul = _bf16_split(u64)
    vh, vl = _bf16_split(v64)
    uv_full = np.concatenate(
        [uh.T[None], vh.T[None], ul.T[None], vl.T[None]], axis=0
    )  # (4, D, N)
    uv_shards = []
    for i in range(NCORES):
        sl = uv_full[:, :, i * NLOC:(i + 1) * NLOC]      # (4, D, NLOC)
        uv_shards.append(np.ascontiguousarray(
            sl.reshape(4 * D, NLOC), ml_dtypes.bfloat16))

    # --- basis-slab selection matrices (FG, 4*D, RP) ------------------
    # slab j holds features 7j..7j+6 (last: 4 features + pad): column
    # q = 17*r + k of slab j selects k*u_f + (16-k)*v_f with f = 7j+r.
    ks = np.arange(K, dtype=np.float64)
    amat = np.zeros((FG, 4 * D, RP), np.float64)
    for j in range(FG):
        A = amat[j]
        for r in range(FPG):
            f = j * FPG + r
            if f >= D:
                continue
            q = slice(K * r, K * r + K)
            A[f, q] = ks
            A[D + f, q] = ORDER - ks
            A[2 * D + f, q] = ks
            A[3 * D + f, q] = ORDER - ks
    amat = amat.astype(ml_dtypes.bfloat16)

    # --- block-diagonal chain weights (bf16) --------------------------
    sc2 = prior_sc[:, 0] ** 2                            # (K,)
    wmean = np.zeros(((D - 1) * G, RP, RP), np.float64)
    wvar = np.zeros(((D - 1) * G, RP, RP), np.float64)
    for d in range(1, D):
        for g in range(G):
            Wm = wmean[(d - 1) * G + g]
            Wv = wvar[(d - 1) * G + g]
            for c in range(CPG):
                p = g * CPG + c
                if p >= P:
                    continue
                blk = slice(K * c, K * c + K)
                m = meanw[d - 1, p]                      # (K, K) [k, j]
                v = np.exp(varw[d - 1, p]) * sc2[None, :]
                if d == 1:
                    m = meanw0[p, 0][:, None] * m
                    v = (np.exp(varw0[p, 0]) * sc2)[:, None] * v
                Wm[blk, blk] = m
                Wv[blk, blk] = v
    wmean = wmean.astype(ml_dtypes.bfloat16)
    wvar = wvar.astype(ml_dtypes.bfloat16)

    # --- reduction vectors (G, RP, 2): col0 mean ones, col1 var 1/pp --
    if np.all(post_prec > 0):
        qbar = float(np.exp(np.mean(np.log(1.0 / post_prec))))
    else:
        qbar = 1.0
    qbar_inv = (1.0 / post_prec) / qbar
    redw = np.zeros((G, RP, 2), np.float64)
    for g in range(G):
        for c in range(CPG):
            p = g * CPG + c
            if p >= P:
                continue
            blk = slice(K * c, K * c + K)
            redw[g, blk, 0] = 1.0
            redw[g, blk, 1] = qbar_inv[p]
    redw = redw.astype(ml_dtypes.bfloat16)

    # --- exp biases: log binom / 2 log binom (per partition) ----------
    logb = np.log(np.array([comb(ORDER, k) for k in range(K)], np.float64))
    biasv = np.zeros((RP, 2), np.float64)
    biasv[:R, 0] = np.tile(logb, CPG)
    biasv[:R, 1] = 2.0 * np.tile(logb, CPG)
    biasv = biasv.astype(np.float32)

    shared = dict(amat=amat, wmean=wmean, wvar=wvar, redw=redw, biasv=biasv)
    return uv_shards, shared, qbar


def _build_module(nloc=NLOC):
    smode = _flags()
    perm = perm_host
    nchunk = max(1, nloc // CHUNK)
    chunk = min(CHUNK, nloc)
    nred = max(1, nloc // HALF)
    rhalf = min(HALF, nloc)
    nh = chunk // rhalf                     # 512-halves per chunk

    nc = bacc.Bacc("TRN2", target_bir_lowering=False, debug=False)
    uv_d = nc.dram_tensor("uv", [4 * D, nloc], BF16, kind="ExternalInput").ap()
    amat_d = nc.dram_tensor("amat", [FG, 4 * D, RP], BF16, kind="ExternalInput").ap()
    wm_d = nc.dram_tensor("wmean", [(D - 1) * G, RP, RP], BF16, kind="ExternalInput").ap()
    wv_d = nc.dram_tensor("wvar", [(D - 1) * G, RP, RP], BF16, kind="ExternalInput").ap()
    red_d = nc.dram_tensor("redw", [G, RP, 2], BF16, kind="ExternalInput").ap()
    bias_d = nc.dram_tensor("biasv", [RP, 2], F32, kind="ExternalInput").ap()
    out_d = nc.dram_tensor("out", [2, nloc], F32, kind="ExternalOutput").ap()

    tiles = [(d, g, ci) for ci in range(nchunk) for d in range(D) for g in range(G)]
    ntile = len(tiles)
    tpc = D * G                              # tiles per chunk

    with tile.TileContext(nc) as tc:
        with (
            tc.tile_pool(name="persist", bufs=1) as persist,
            tc.tile_pool(name="wpool", bufs=4) as wpool,
            tc.tile_pool(name="slabs", bufs=2) as slabs,
            tc.tile_pool(name="mpool", bufs=4) as mpool,
            tc.tile_pool(name="cvpool", bufs=4) as cvpool,
            tc.tile_pool(name="psB", bufs=1, space="PSUM") as psB,
            tc.tile_pool(name="psC", bufs=3, space="PSUM") as psC,
        ):
            uv = persist.tile([4 * D, nloc], BF16, tag="uv")
            for ci in range(nchunk):
                nc.sync.dma_start(
                    uv[:, ci * chunk:(ci + 1) * chunk],
                    uv_d[:, ci * chunk:(ci + 1) * chunk])
            bias = persist.tile([RP, 2], F32, tag="bias")
            nc.sync.dma_start(bias[:], bias_d)
            amat_t = []
            for j in range(FG):
                a = persist.tile([4 * D, RP], BF16, tag=f"A{j}")
                nc.sync.dma_start(a[:], amat_d[j])
                amat_t.append(a)
            states = []
            for g in range(G):
                s = persist.tile([RP, nchunk, 2, chunk], BF16, tag=f"S{g}")
                states.append(s)
            redt = []
            for g in range(G):
                r = persist.tile([RP, 2], BF16, tag=f"RW{g}")
                nc.sync.dma_start(r[:], red_d[g])
                redt.append(r)

            # ---------------- basis slab build (per chunk) ------------
            # slab tile: [RP, 2, chunk]; [:,0,:] = B, [:,1,:] = B^2
            built = {}           # ci -> list of slab tiles (lazily filled)

            def build_slab(ci, j):
                if ci >= nchunk:
                    return
                bts = built.setdefault(ci, [None] * FG)
                if bts[j] is not None:
                    return
                c0 = ci * chunk
                bt = slabs.tile([RP, 2, chunk], BF16, tag=f"B{j}")
                ps = psB.tile([RP, chunk], F32, tag="LB")
                for h in range(nh):
                    nc.tensor.matmul(
                        ps[:, h * rhalf:(h + 1) * rhalf],
                        amat_t[j][:],
                        uv[:, c0 + h * rhalf:c0 + (h + 1) * rhalf],
                        start=True, stop=True)
                nc.scalar.activation(
                    bt[:, 0, :], ps[:], EXP, bias=bias[:, 0:1], scale=1.0)
                nc.scalar.activation(
                    bt[:, 1, :], ps[:], EXP, bias=bias[:, 1:2], scale=2.0)
                bts[j] = bt

            def build_slabs(ci):
                for j in range(FG):
                    build_slab(ci, j)

            # DMA-gather of the per-tile multiplier [7c x 17k, 2, n].
            dmae = [nc.sync]
            dmac = [0]

            def gather_m(dst, d, g, ci):
                bts = built[ci]
                for c in range(CPG):
                    p = g * CPG + c
                    if p >= P:
                        continue
                    f = int(perm[p, d])
                    j, r = divmod(f, FPG)
                    src = bts[j][K * r:K * r + K, :, :]
                    eng = dmae[dmac[0] % len(dmae)]
                    dmac[0] += 1
                    eng.dma_start(dst[K * c:K * c + K], src)

            loaded = {}

            def ensure_dg(t):
                if t >= ntile:
                    return
                d, g, _ci = tiles[t]
                if d < 1:
                    return
                dg = (d - 1) * G + g
                if dg in loaded:
                    return
                wm_t = wpool.tile([RP, RP], BF16, tag="WM")
                nc.sync.dma_start(wm_t[:], wm_d[dg])
                wv_t = wpool.tile([RP, RP], BF16, tag="WV")
                nc.sync.dma_start(wv_t[:], wv_d[dg])
                loaded[dg] = (wm_t, wv_t)

            mstore = {}

            def emit_gather(t):
                if t >= ntile:
                    return
                d, g, ci = tiles[t]
                if d == 0:
                    # gather B/B^2 straight into the chain state
                    gather_m(states[g][:, ci, :, :], 0, g, ci)
                else:
                    m_t = mpool.tile([RP, 2, chunk], BF16, tag="M")
                    mstore[t] = m_t
                    gather_m(m_t, d, g, ci)

            def emit_compute(t):
                d, g, ci = tiles[t]
                if d == 0:
                    return
                m_t = mstore.pop(t)
                S = states[g]
                wm_t, wv_t = loaded[(d - 1) * G + g]
                split = (t % 3) < smode
                for h in range(nh):
                    hs = slice(h * rhalf, (h + 1) * rhalf)
                    pc = psC.tile([RP, 2, rhalf], F32, tag="C")
                    nc.tensor.matmul(pc[:, 1, :], wv_t[:], S[:, ci, 1, hs],
                                     start=True, stop=True)
                    nc.tensor.matmul(pc[:, 0, :], wm_t[:], S[:, ci, 0, hs],
                                     start=True, stop=True)
                    if split:
                        # ACT evacuates the var chain; GPSIMD multiplies
                        cv = cvpool.tile([RP, rhalf], BF16, tag="CV")
                        nc.scalar.copy(cv[:], pc[:, 1, :])
                        nc.gpsimd.tensor_tensor(
                            S[:, ci, 1, hs], cv[:], m_t[:, 1, hs], MULT)
                        nc.vector.tensor_tensor(
                            S[:, ci, 0, hs], pc[:, 0, :], m_t[:, 0, hs], MULT)
                    else:
                        nc.vector.tensor_tensor(
                            S[:, ci, :, hs], pc[:], m_t[:, :, hs], MULT)

            # ---------------- software-pipelined emission -------------
            build_slabs(0)
            ensure_dg(0)
            emit_gather(0)
            emit_gather(1)
            for t in range(ntile):
                d, g, ci = tiles[t]
                if d == 0 and g == 0:
                    loaded.clear()       # weight ring slots are per-chunk
                ensure_dg(t + 1)
                ensure_dg(t + 2)
                emit_gather(t + 2)
                # spread next chunk's slab builds across this chunk
                tc_pos = t % tpc
                if tc_pos >= 30 and tc_pos < 30 + 10 * FG and (tc_pos - 30) % 10 == 0:
                    build_slab(t // tpc + 1, (tc_pos - 30) // 10)
                emit_compute(t)

            # ---- final reduction: sum over (chain, k) partitions -----
            outs = persist.tile([1, 2 * nloc], F32, tag="outs")
            for ci in range(nred):
                o0 = ci * rhalf
                cc, off = divmod(o0, chunk)
                prt = psC.tile([RP, 2, rhalf], F32, tag="C")
                pr = prt[0:1]
                for g in range(G):
                    nc.tensor.matmul(
                        pr[:, 0, :], redt[g][:, 0:1],
                        states[g][:, cc, 0, off:off + rhalf],
                        start=(g == 0), stop=(g == G - 1))
                for g in range(G):
                    nc.tensor.matmul(
                        pr[:, 1, :], redt[g][:, 1:2],
                        states[g][:, cc, 1, off:off + rhalf],
                        start=(g == 0), stop=(g == G - 1))
                nc.scalar.copy(outs[0:1, o0:o0 + rhalf], pr[:, 0, :])
                nc.scalar.copy(
                    outs[0:1, nloc + o0:nloc + o0 + rhalf], pr[:, 1, :])
            nc.sync.dma_start(out_d.rearrange("a b -> (a b)")[None, :], outs[:])

    nc.compile()
    return nc


def kernel(Xnew, meanw0, meanw, varw0, varw, prior_sc, post_prec, perm):
    uv_shards, shared, qbar = _host_tensors(
        Xnew, meanw0, meanw, varw0, varw, prior_sc, post_prec, perm)
    nc = _build_module(NLOC)
    in_maps = [dict(uv=uv_shards[i], **shared) for i in range(NCORES)]
    res = bass_utils.run_bass_kernel_spmd(
        nc, in_maps, core_ids=list(range(NCORES)))
    outs = [res.results[i]["out"] for i in range(NCORES)]
    f_mean = np.concatenate([o[0] for o in outs]).reshape(N, 1).astype(np.float32)
    f_var = (np.concatenate([o[1] for o in outs]).reshape(N, 1)
             * np.float32(qbar)).astype(np.float32)
    return f_mean, f_var


# revision 6
# speedup vs baseline: 5.7977x; 5.7977x over previous
"""Trainium2 Bass kernel for nn_BezierButtress (Bernstein-basis permutation chains).

Math (per permutation chain p, over depth d = 0..31):
    S_mean <- (S_mean @ Wm_d) * B(x_{perm[p,d]})        (K=17 wide state)
    S_var  <- (S_var  @ Wv_d) * B(x_{perm[p,d]})^2
    outputs: f_mean[n] = sum_{p,k} S_mean, f_var[n] = sum_{p,k} S_var / post_prec[p]

Device strategy (data-parallel over N across 8 cores, 3072 rows each):
  * all matmuls bf16 (PE streams 2x faster than fp32r); chain states,
    weights and multipliers bf16 -- rel-err gate is 2e-2, bf16 lands ~5e-3.
  * the per-step multipliers B / B^2 are fully precomputed ON THE HOST in
    fp64 and laid out in HBM per (d, group) tile: [96, 128, 2, nloc] bf16.
    The device just streams them in with one DMA per tile -- no gather
    matmul, no exp, no squaring on the device at all.  This frees the
    Scalar and GpSimd engines entirely for draining PSUM.
  * chain weights (93 x 2 block-diagonal 128x128 bf16) persist in SBUF,
    loaded with two bulk DMAs at startup.
  * the PSUM drain is the binding resource (PSUM reads are 1 elem/cycle
    per engine): every tile the DVE multiplies the mean chain out of
    PSUM while ACT copies the var chain to SBUF (bf16); the var multiply
    then runs on GPSIMD (2 of 3 tiles) or on the DVE in its fast 2x bf16
    SBUF mode (1 of 3 tiles).
  * d=0 initializes states directly from the multiplier table (meanw0 /
    exp(varw0)*sc2 prefactors are folded into the d=1 weights); partition
    rows 119..127 are zeroed once so the block-diagonal matmuls contract
    exact zeros.
"""

import os
import numpy as np
import ml_dtypes
from math import comb

import concourse.bass as bass
import concourse.mybir as mybir
import concourse.tile as tile
from concourse import bacc
from concourse import bass_utils

ORDER = 16
K = 17
D = 32
P = 20
N = 24576
NCORES = 8
NLOC = N // NCORES        # 3072
CPG = 7                   # chain slots per group
G = 3                     # groups (7, 7, 6 + 1 pad)
R = CPG * K               # 119 active partitions
RP = 128                  # padded partition count
CHUNK = 1024
HALF = 512
F32 = mybir.dt.float32
BF16 = mybir.dt.bfloat16
MULT = mybir.AluOpType.mult


def _flags():
    # BB_VMOD: every BB_VMOD-th tile runs the var multiply on the DVE
    # (2x bf16 SBUF mode) instead of GPSIMD.  0 disables DVE var tiles.
    vmod = int(os.environ.get("BB_VMOD", "3"))
    return vmod


def _host_tensors(Xnew, meanw0, meanw, varw0, varw, prior_sc, post_prec, perm):
    Xnew = np.asarray(Xnew, np.float64)       # (N, D)
    meanw0 = np.asarray(meanw0, np.float64)   # (P, 1, K)
    meanw = np.asarray(meanw, np.float64)     # (D-1, P, K, K)
    varw0 = np.asarray(varw0, np.float64)     # (P, 1, K)
    varw = np.asarray(varw, np.float64)       # (D-1, P, K, K)
    prior_sc = np.asarray(prior_sc, np.float64)  # (K, 1)
    post_prec = np.asarray(post_prec, np.float64)  # (P,)
    perm = np.asarray(perm)                   # (P, D) int

    # --- Bernstein basis for every (n, feature, k), fp64 --------------
    ks = np.arange(K, dtype=np.float64)
    binom = np.array([comb(ORDER, k) for k in range(K)], np.float64)
    Xe = Xnew[..., None]                      # (N, D, 1)
    Bnk = (Xe ** ks) * ((1.0 - Xe) ** (ORDER - ks)) * binom   # (N, D, K)

    # --- per-tile multiplier table [D*G, RP, 2, N] bf16 ---------------
    mtab = np.ones((D * G, RP, 2, N), ml_dtypes.bfloat16)
    for d in range(D):
        for g in range(G):
            t = mtab[d * G + g]
            for c in range(CPG):
                p = g * CPG + c
                if p >= P:
                    continue
                b = Bnk[:, perm[p, d], :]                    # (N, K)
                t[K * c:K * c + K, 0, :] = b.T.astype(ml_dtypes.bfloat16)
                t[K * c:K * c + K, 1, :] = (b * b).T.astype(ml_dtypes.bfloat16)
    mtab_shards = [
        np.ascontiguousarray(mtab[:, :, :, i * NLOC:(i + 1) * NLOC])
        for i in range(NCORES)
    ]

    # --- block-diagonal chain weights (bf16) --------------------------
    sc2 = prior_sc[:, 0] ** 2                            # (K,)
    wmean = np.zeros(((D - 1) * G, RP, RP), np.float64)
    wvar = np.zeros(((D - 1) * G, RP, RP), np.float64)
    for d in range(1, D):
        for g in range(G):
            Wm = wmean[(d - 1) * G + g]
            Wv = wvar[(d - 1) * G + g]
            for c in range(CPG):
                p = g * CPG + c
                if p >= P:
                    continue
                blk = slice(K * c, K * c + K)
                m = meanw[d - 1, p]                      # (K, K) [k, j]
                v = np.exp(varw[d - 1, p]) * sc2[None, :]
                if d == 1:
                    m = meanw0[p, 0][:, None] * m
                    v = (np.exp(varw0[p, 0]) * sc2)[:, None] * v
                Wm[blk, blk] = m
                Wv[blk, blk] = v
    wmean = wmean.astype(ml_dtypes.bfloat16)
    wvar = wvar.astype(ml_dtypes.bfloat16)

    # --- reduction vectors (G, RP, 2): col0 mean ones, col1 var 1/pp --
    if np.all(post_prec > 0):
        qbar = float(np.exp(np.mean(np.log(1.0 / post_prec))))
    else:
        qbar = 1.0
    qbar_inv = (1.0 / post_prec) / qbar
    redw = np.zeros((G, RP, 2), np.float64)
    for g in range(G):
        for c in range(CPG):
            p = g * CPG + c
            if p >= P:
                continue
            blk = slice(K * c, K * c + K)
            redw[g, blk, 0] = 1.0
            redw[g, blk, 1] = qbar_inv[p]
    redw = redw.astype(ml_dtypes.bfloat16)

    shared = dict(wmean=wmean, wvar=wvar, redw=redw)
    return mtab_shards, shared, qbar


def _build_module(nloc=NLOC):
    vmod = _flags()
    nchunk = max(1, nloc // CHUNK)
    chunk = min(CHUNK, nloc)
    nred = max(1, nloc // HALF)
    rhalf = min(HALF, nloc)
    nh = chunk // rhalf                     # 512-halves per chunk

    nc = bacc.Bacc("TRN2", target_bir_lowering=False, debug=False)
    mt_d = nc.dram_tensor("uv", [D * G, RP, 2, nloc], BF16, kind="ExternalInput").ap()
    wm_d = nc.dram_tensor("wmean", [(D - 1) * G, RP, RP], BF16, kind="ExternalInput").ap()
    wv_d = nc.dram_tensor("wvar", [(D - 1) * G, RP, RP], BF16, kind="ExternalInput").ap()
    red_d = nc.dram_tensor("redw", [G, RP, 2], BF16, kind="ExternalInput").ap()
    out_d = nc.dram_tensor("out", [2, nloc], F32, kind="ExternalOutput").ap()

    tiles = [(d, g, ci) for ci in range(nchunk) for d in range(D) for g in range(G)]
    ntile = len(tiles)

    with tile.TileContext(nc) as tc:
        with (
            tc.tile_pool(name="persist", bufs=1) as persist,
            tc.tile_pool(name="mpool", bufs=6) as mpool,
            tc.tile_pool(name="cvpool", bufs=4) as cvpool,
            tc.tile_pool(name="psC", bufs=4, space="PSUM") as psC,
        ):
            # chain weights persist in SBUF: [RP, 93, RP] per kind
            wm = persist.tile([RP, (D - 1) * G, RP], BF16, tag="WM")
            nc.sync.dma_start(wm[:], wm_d.rearrange("a b c -> b a c"))
            wv = persist.tile([RP, (D - 1) * G, RP], BF16, tag="WV")
            nc.sync.dma_start(wv[:], wv_d.rearrange("a b c -> b a c"))
            states = []
            for g in range(G):
                s = persist.tile([RP, nchunk, 2, chunk], BF16, tag=f"S{g}")
                # pad partitions must hold finite zeros: the block-diag
                # weights have zero columns there, and 0 * garbage-NaN
                # would poison the contraction.
                nc.vector.memset(s[96:128], 0.0)
                states.append(s)
            redt = []
            for g in range(G):
                r = persist.tile([RP, 2], BF16, tag=f"RW{g}")
                nc.sync.dma_start(r[:], red_d[g])
                redt.append(r)

            mstore = {}

            def emit_load(t):
                if t >= ntile:
                    return
                d, g, ci = tiles[t]
                c0 = ci * chunk
                if d == 0:
                    nc.sync.dma_start(
                        states[g][0:R, ci, :, :],
                        mt_d[g, 0:R, :, c0:c0 + chunk])
                else:
                    m_t = mpool.tile([RP, 2, chunk], BF16, tag="M")
                    mstore[t] = m_t
                    nc.sync.dma_start(
                        m_t[0:R], mt_d[d * G + g, 0:R, :, c0:c0 + chunk])

            def emit_compute(t):
                d, g, ci = tiles[t]
                if d == 0:
                    return
                m_t = mstore.pop(t)
                S = states[g]
                dg = (d - 1) * G + g
                var_on_dve = vmod and (t % vmod == vmod - 1)
                for h in range(nh):
                    hs = slice(h * rhalf, (h + 1) * rhalf)
                    pc = psC.tile([RP, 2, rhalf], F32, tag="C")
                    nc.tensor.matmul(pc[:, 1, :], wv[:, dg, :], S[:, ci, 1, hs],
                                     start=True, stop=True)
                    nc.tensor.matmul(pc[:, 0, :], wm[:, dg, :], S[:, ci, 0, hs],
                                     start=True, stop=True)
                    # ACT evacuates the var chain to SBUF (bf16) ...
                    cv = cvpool.tile([RP, rhalf], BF16, tag="CV")
                    nc.scalar.copy(cv[0:R], pc[0:R, 1, :])
                    # ... DVE drains the mean chain out of PSUM ...
                    nc.vector.tensor_tensor(
                        S[0:R, ci, 0, hs], pc[0:R, 0, :], m_t[0:R, 0, hs], MULT)
                    # ... and the var multiply runs wherever there's room
                    if var_on_dve:
                        nc.vector.tensor_tensor(
                            S[0:R, ci, 1, hs], cv[0:R], m_t[0:R, 1, hs], MULT)
                    else:
                        nc.gpsimd.tensor_tensor(
                            S[0:R, ci, 1, hs], cv[0:R], m_t[0:R, 1, hs], MULT)

            # ---------------- software-pipelined emission -------------
            for t in range(3):
                emit_load(t)
            for t in range(ntile):
                emit_load(t + 3)
                emit_compute(t)

            # ---- final reduction: sum over (chain, k) partitions -----
            outs = persist.tile([1, 2 * nloc], F32, tag="outs")
            for ci in range(nred):
                o0 = ci * rhalf
                cc, off = divmod(o0, chunk)
                prt = psC.tile([RP, 2, rhalf], F32, tag="C")
                pr = prt[0:1]
                for g in range(G):
                    nc.tensor.matmul(
                        pr[:, 0, :], redt[g][:, 0:1],
                        states[g][:, cc, 0, off:off + rhalf],
                        start=(g == 0), stop=(g == G - 1))
                for g in range(G):
                    nc.tensor.matmul(
                        pr[:, 1, :], redt[g][:, 1:2],
                        states[g][:, cc, 1, off:off + rhalf],
                        start=(g == 0), stop=(g == G - 1))
                nc.scalar.copy(outs[0:1, o0:o0 + rhalf], pr[:, 0, :])
                nc.scalar.copy(
                    outs[0:1, nloc + o0:nloc + o0 + rhalf], pr[:, 1, :])
            nc.sync.dma_start(out_d.rearrange("a b -> (a b)")[None, :], outs[:])

    nc.compile()
    return nc


def kernel(Xnew, meanw0, meanw, varw0, varw, prior_sc, post_prec, perm):
    mtab_shards, shared, qbar = _host_tensors(
        Xnew, meanw0, meanw, varw0, varw, prior_sc, post_prec, perm)
    nc = _build_module(NLOC)
    in_maps = [dict(uv=mtab_shards[i], **shared) for i in range(NCORES)]
    res = bass_utils.run_bass_kernel_spmd(
        nc, in_maps, core_ids=list(range(NCORES)))
    outs = [res.results[i]["out"] for i in range(NCORES)]
    f_mean = np.concatenate([o[0] for o in outs]).reshape(N, 1).astype(np.float32)
    f_var = (np.concatenate([o[1] for o in outs]).reshape(N, 1)
             * np.float32(qbar)).astype(np.float32)
    return f_mean, f_var


# revision 18
# speedup vs baseline: 7.5180x; 1.2967x over previous
"""Trainium2 Bass kernel for nn_BezierButtress (Bernstein-basis permutation chains).

Math (per permutation chain p, over depth d = 0..31):
    S_mean <- (S_mean @ Wm_d) * B(x_{perm[p,d]})        (K=17 wide state)
    S_var  <- (S_var  @ Wv_d) * B(x_{perm[p,d]})^2
    outputs: f_mean[n] = sum_{p,k} S_mean, f_var[n] = sum_{p,k} S_var / post_prec[p]

Device strategy (data-parallel over N across 8 cores, 3072 rows each):
  * state layout: (7 chains x 17 k -> 128 partitions incl. pad, n free),
    block-diagonal 128x128 fp32r chain matmuls (3 groups cover 20 chains).
  * per-step Bernstein multipliers built in log space: one PE matmul contracts
    a baked selection/coefficient matrix A_{d,g} (128 x 128) against a resident
    log-table UV (U_hi/V_hi/U_lo/V_lo, 128 x n) giving
    logM = k*log(x_c) + (16-k)*log(1-x_c) exactly (hi/lo splitting cancels the
    PE fp22 truncation); then ACT computes exp(logM + log binom); the squared
    multiplier comes from exp(scale=2) on ACT or an SBUF square on GPSIMD.
  * meanw0 / exp(varw0)*sc2 / sc2 column scale / 1/post_prec are all folded
    host-side into the baked block-diagonal weights & reduction vectors
    (weights pre-rounded to e10m11 so the PE fp32r truncation is a no-op).
  * emission is software-pipelined one tile ahead (gather of tile t+1 before
    compute of tile t).  PSUM is split into a double-buffered logM pool and a
    double-buffered 512-wide chain-output pool so the chain matmuls never wait
    on the exps and the steady state is bound only by the DVE multiplies.
"""

import os
import numpy as np
import ml_dtypes
from math import comb

import concourse.bass as bass
import concourse.mybir as mybir
import concourse.tile as tile
from concourse import bacc
from concourse import bass_utils

ORDER = 16
K = 17
D = 32
P = 20
N = 24576
NCORES = 8
NLOC = N // NCORES        # 3072
CPG = 7                   # chain slots per group
G = 3                     # groups (7, 7, 6 + 1 pad)
R = CPG * K               # 119 active partitions
RP = 128                  # padded partition count
CHUNK = 1024
HALF = 512
F32 = mybir.dt.float32
F32R = mybir.dt.float32r
BF16 = mybir.dt.bfloat16
EXP = mybir.ActivationFunctionType.Exp
MULT = mybir.AluOpType.mult


def _flags():
    # NOTE: walrus rejects mixed 32-bit/16-bit matmul inputs (NCC_IBIR034),
    # so bf16 operands require BOTH sides bf16.  a16=2 runs the whole gather
    # matmul (A and the UV log-table) in bf16 -- A entries are small exact
    # integers and UV is hi/lo split, so the effective log-table mantissa is
    # ~16 bits; bf16 streams ~2x faster through the PE than fp32r.
    a16 = int(os.environ.get("BB_A16", "0"))         # 0=f32r, 2=bf16 gather
    w16 = bool(int(os.environ.get("BB_W16", "0")))   # bf16 hi/lo weights (invalid)
    gp3 = int(os.environ.get("BB_GP3", "2"))         # GP square 2-of-3 tiles
    mulmod = int(os.environ.get("BB_MULMOD", "0"))   # 1-in-mulmod muls via ACT+GP
    return a16, w16, gp3, mulmod


def _fp22_round(x64):
    """Round float64 to the nearest fp22 (e10m11) value, returned as float32.
    The PE's fp32r path *truncates* inputs to fp22; feeding it pre-rounded
    values makes that truncation a no-op and kills the systematic bias."""
    x32 = x64.astype(np.float32)
    u = x32.view(np.uint32).astype(np.uint64)
    u = ((u + 0x800) & 0xFFFFF000).astype(np.uint32)   # round-half-up on m11
    return u.view(np.float32)


def _fp22_split(x64):
    """Split float64 -> (hi, lo) float32 with hi exactly representable in
    fp22 (e10m11), so PE fp32r matmuls consume hi/lo exactly."""
    x32 = x64.astype(np.float32)
    hi = (x32.view(np.uint32) & np.uint32(0xFFFFF000)).view(np.float32)
    lo = (x64 - hi.astype(np.float64)).astype(np.float32)
    return hi, lo


def _bf16_split(x64):
    hi = x64.astype(ml_dtypes.bfloat16)
    lo = (x64 - hi.astype(np.float64)).astype(ml_dtypes.bfloat16)
    return hi, lo


def _host_tensors(Xnew, meanw0, meanw, varw0, varw, prior_sc, post_prec, perm):
    a16, w16, _, _ = _flags()
    Xnew = np.asarray(Xnew, np.float32)
    meanw0 = np.asarray(meanw0, np.float64)   # (P, 1, K)
    meanw = np.asarray(meanw, np.float64)     # (D-1, P, K, K)
    varw0 = np.asarray(varw0, np.float64)     # (P, 1, K)
    varw = np.asarray(varw, np.float64)       # (D-1, P, K, K)
    prior_sc = np.asarray(prior_sc, np.float64)  # (K, 1)
    post_prec = np.asarray(post_prec, np.float64)  # (P,)
    perm = np.asarray(perm)                   # (P, D) int

    # --- per-core UV log tables ---------------------------------------
    x64 = np.clip(Xnew.astype(np.float64), 1e-30, None)
    u64 = np.log(x64)                                    # (N, D)
    v64 = np.log1p(-np.minimum(Xnew.astype(np.float64), 1.0 - 1e-15))
    split = _bf16_split if a16 == 2 else _fp22_split
    uv_np_dt = ml_dtypes.bfloat16 if a16 == 2 else np.float32
    uh, ul = split(u64)
    vh, vl = split(v64)
    uv_full = np.concatenate(
        [uh.T[None], vh.T[None], ul.T[None], vl.T[None]], axis=0
    )  # (4, D, N)
    uv_shards = []
    for i in range(NCORES):
        sl = uv_full[:, :, i * NLOC:(i + 1) * NLOC]      # (4, D, NLOC)
        uv_shards.append(np.ascontiguousarray(sl.reshape(4 * D, NLOC), uv_np_dt))

    # --- A selection/coefficient matrices (D*G, 128, RP) --------------
    ks = np.arange(K, dtype=np.float64)
    amat = np.zeros((D * G, 4 * D, RP), np.float64)
    for d in range(D):
        for g in range(G):
            A = amat[d * G + g]
            for c in range(CPG):
                p = g * CPG + c
                if p >= P:
                    continue
                col = perm[p, d]
                j = slice(K * c, K * c + K)
                A[col, j] = ks
                A[D + col, j] = ORDER - ks
                A[2 * D + col, j] = ks
                A[3 * D + col, j] = ORDER - ks
    amat = amat.astype(ml_dtypes.bfloat16) if a16 else amat.astype(np.float32)

    # --- block-diagonal chain weights ---------------------------------
    sc2 = prior_sc[:, 0] ** 2                            # (K,)
    wmean = np.zeros(((D - 1) * G, RP, RP), np.float64)
    wvar = np.zeros(((D - 1) * G, RP, RP), np.float64)
    for d in range(1, D):
        for g in range(G):
            Wm = wmean[(d - 1) * G + g]
            Wv = wvar[(d - 1) * G + g]
            for c in range(CPG):
                p = g * CPG + c
                if p >= P:
                    continue
                blk = slice(K * c, K * c + K)
                m = meanw[d - 1, p]                      # (K, K) [k, j]
                v = np.exp(varw[d - 1, p]) * sc2[None, :]
                if d == 1:
                    m = meanw0[p, 0][:, None] * m
                    v = (np.exp(varw0[p, 0]) * sc2)[:, None] * v
                Wm[blk, blk] = m
                Wv[blk, blk] = v
    if w16:
        wmh, wml = _bf16_split(wmean)
        wvh, wvl = _bf16_split(wvar)
        wmean = np.stack([wmh, wml], axis=1)             # (93, 2, RP, RP)
        wvar = np.stack([wvh, wvl], axis=1)
    else:
        wmean = _fp22_round(wmean)
        wvar = _fp22_round(wvar)

    # --- reduction vectors (G, RP, 2): col0 mean ones, col1 var 1/pp --
    # factor the geometric-mean scale of 1/post_prec out to the host so the
    # device-side values are ~1 (exactly 1 for uniform post_prec: no rounding)
    if np.all(post_prec > 0):
        qbar = float(np.exp(np.mean(np.log(1.0 / post_prec))))
    else:
        qbar = 1.0
    qbar_inv = (1.0 / post_prec) / qbar
    redw = np.zeros((G, RP, 2), np.float64)
    for g in range(G):
        for c in range(CPG):
            p = g * CPG + c
            if p >= P:
                continue
            blk = slice(K * c, K * c + K)
            redw[g, blk, 0] = 1.0
            redw[g, blk, 1] = qbar_inv[p]
    redw = _fp22_round(redw)

    # --- exp biases: log binom / 2 log binom (per partition) ----------
    logb = np.log(np.array([comb(ORDER, k) for k in range(K)], np.float64))
    biasv = np.zeros((RP, 2), np.float64)
    biasv[:R, 0] = np.tile(logb, CPG)
    biasv[:R, 1] = 2.0 * np.tile(logb, CPG)
    biasv = biasv.astype(np.float32)

    shared = dict(amat=amat, wmean=wmean, wvar=wvar, redw=redw, biasv=biasv)
    return uv_shards, shared, qbar


def _build_module(nloc=NLOC):
    a16, w16, gp3, mulmod = _flags()
    nchunk = max(1, nloc // CHUNK)
    chunk = min(CHUNK, nloc)
    nred = max(1, nloc // HALF)
    rhalf = min(HALF, nloc)
    nh = chunk // rhalf                     # 512-halves per chunk

    A_DT = BF16 if a16 else F32R
    UV_DT = BF16 if a16 == 2 else F32R
    W_DT = BF16 if w16 else F32R
    wshape = [2, RP, RP] if w16 else [RP, RP]

    nc = bacc.Bacc("TRN2", target_bir_lowering=False, debug=False)
    uv_d = nc.dram_tensor("uv", [4 * D, nloc], UV_DT, kind="ExternalInput").ap()
    amat_d = nc.dram_tensor("amat", [D * G, 4 * D, RP], A_DT, kind="ExternalInput").ap()
    wm_d = nc.dram_tensor("wmean", [(D - 1) * G] + wshape, W_DT, kind="ExternalInput").ap()
    wv_d = nc.dram_tensor("wvar", [(D - 1) * G] + wshape, W_DT, kind="ExternalInput").ap()
    red_d = nc.dram_tensor("redw", [G, RP, 2], F32R, kind="ExternalInput").ap()
    bias_d = nc.dram_tensor("biasv", [RP, 2], F32, kind="ExternalInput").ap()
    out_d = nc.dram_tensor("out", [2, nloc], F32, kind="ExternalOutput").ap()

    tiles = [(d, g, ci) for d in range(D) for g in range(G) for ci in range(nchunk)]
    ntile = len(tiles)

    with tile.TileContext(nc) as tc:
        with (
            tc.tile_pool(name="persist", bufs=1) as persist,
            tc.tile_pool(name="wpool", bufs=4) as wpool,
            tc.tile_pool(name="mpool", bufs=4) as mpool,
            tc.tile_pool(name="psL", bufs=int(os.environ.get("BB_PSL", "2")), space="PSUM") as psL,
            tc.tile_pool(name="psC", bufs=int(os.environ.get("BB_PSC", "2")), space="PSUM") as psC,
        ):
            uv = persist.tile([4 * D, nloc], UV_DT, tag="uv")
            # per-chunk pieces so the first gather starts sooner
            for ci in range(nchunk):
                nc.sync.dma_start(
                    uv[:, ci * chunk:(ci + 1) * chunk],
                    uv_d[:, ci * chunk:(ci + 1) * chunk])
            bias = persist.tile([RP, 2], F32, tag="bias")
            nc.sync.dma_start(bias[:], bias_d)
            states = []
            for g in range(G):
                s = persist.tile([RP, nchunk, 2, chunk], F32R, tag=f"S{g}")
                states.append(s)
            redt = []
            for g in range(G):
                r = persist.tile([RP, 2], F32R, tag=f"RW{g}")
                nc.sync.dma_start(r[:], red_d[g])
                redt.append(r)

            loaded = {}

            def ensure_dg(t):
                if t >= ntile:
                    return
                d, g, _ = tiles[t]
                dg = d * G + g
                if dg in loaded:
                    return
                a_t = wpool.tile([4 * D, RP], A_DT, tag="A")
                nc.sync.dma_start(a_t[:], amat_d[dg])
                entry = {"A": a_t}
                if d >= 1:
                    wm_t = wpool.tile(wshape, W_DT, tag="WM")
                    nc.sync.dma_start(wm_t[:], wm_d[(d - 1) * G + g])
                    wv_t = wpool.tile(wshape, W_DT, tag="WV")
                    nc.sync.dma_start(wv_t[:], wv_d[(d - 1) * G + g])
                    entry["WM"] = wm_t
                    entry["WV"] = wv_t
                loaded[dg] = entry

            pstore = {}

            def emit_gather(t):
                d, g, ci = tiles[t]
                a_t = loaded[d * G + g]["A"]
                ps = psL.tile([RP, chunk], F32, tag="L")
                pstore[t] = ps
                c0 = ci * chunk
                for h in range(nh):
                    nc.tensor.matmul(
                        ps[:, h * rhalf:(h + 1) * rhalf],
                        a_t[:],
                        uv[:, c0 + h * rhalf:c0 + (h + 1) * rhalf],
                        start=True, stop=True)

            def emit_compute(t):
                d, g, ci = tiles[t]
                ps = pstore.pop(t)
                S = states[g]
                use_gp = (t % 3) < gp3
                if d == 0:
                    # initial states are the multipliers themselves
                    # (meanw0 / varw0 prefactors folded into d=1 weights);
                    # the square runs on the otherwise-idle DVE at startup
                    nc.scalar.activation(
                        S[:, ci, 0, :], ps[:], EXP,
                        bias=bias[:, 0:1], scale=1.0)
                    nc.vector.tensor_tensor(
                        S[:, ci, 1, :], S[:, ci, 0, :], S[:, ci, 0, :], MULT)
                    return
                ent = loaded[d * G + g]
                m_t = mpool.tile([RP, 2, chunk], F32, tag="M")
                nc.scalar.activation(
                    m_t[:, 0, :], ps[:], EXP, bias=bias[:, 0:1], scale=1.0)
                if use_gp:
                    nc.gpsimd.tensor_tensor(
                        m_t[:, 1, :], m_t[:, 0, :], m_t[:, 0, :], MULT)
                else:
                    nc.scalar.activation(
                        m_t[:, 1, :], ps[:], EXP, bias=bias[:, 1:2], scale=2.0)
                # chain matmuls live in their own small PSUM tiles so they
                # never wait on the exps; the DVE mul is the only consumer
                # of both streams
                c0 = ci * chunk
                for h in range(nh):
                    hs = slice(h * rhalf, (h + 1) * rhalf)
                    pc = psC.tile([RP, 2, rhalf], F32, tag="C")
                    for trow, wkey in ((1, "WV"), (0, "WM")):
                        w_t = ent[wkey]
                        dst = pc[:, trow, :]
                        src = S[:, ci, trow, hs]
                        if w16:
                            nc.tensor.matmul(dst, w_t[0], src, start=True, stop=False)
                            nc.tensor.matmul(dst, w_t[1], src, start=False, stop=True)
                        else:
                            nc.tensor.matmul(dst, w_t[:], src, start=True, stop=True)
                    if mulmod and (t * nh + h) % mulmod == 0:
                        # offload this multiply: ACT evacuates the chain
                        # PSUM to SBUF, GPSIMD does the multiply (GPSIMD
                        # cannot read PSUM directly)
                        sb = mpool.tile([RP, 2, rhalf], F32, tag="B")
                        nc.scalar.copy(sb[:], pc[:])
                        nc.gpsimd.tensor_tensor(
                            S[:, ci, :, hs], sb[:], m_t[:, :, hs], MULT)
                    else:
                        nc.vector.tensor_tensor(
                            S[:, ci, :, hs], pc[:], m_t[:, :, hs], MULT)

            # software-pipelined emission: gather one tile ahead
            ensure_dg(0)
            emit_gather(0)
            for t in range(ntile):
                ensure_dg(t + 1)
                ensure_dg(t + nchunk + 1)    # prefetch next (d,g) weights
                if t + 1 < ntile:
                    emit_gather(t + 1)
                emit_compute(t)

            # ---- final reduction: sum over (chain, k) partitions -----
            # single partition row: [mean(nloc) | var(nloc)] (engine APs
            # must start on quadrant-aligned partitions, so no row 1)
            outs = persist.tile([1, 2 * nloc], F32, tag="outs")
            for ci in range(nred):
                o0 = ci * rhalf
                cc, off = divmod(o0, chunk)
                pr = psC.tile([1, 2, rhalf], F32, tag="C")
                for g in range(G):
                    nc.tensor.matmul(
                        pr[:, 0, :], redt[g][:, 0:1],
                        states[g][:, cc, 0, off:off + rhalf],
                        start=(g == 0), stop=(g == G - 1))
                for g in range(G):
                    nc.tensor.matmul(
                        pr[:, 1, :], redt[g][:, 1:2],
                        states[g][:, cc, 1, off:off + rhalf],
                        start=(g == 0), stop=(g == G - 1))
                nc.scalar.copy(outs[0:1, o0:o0 + rhalf], pr[:, 0, :])
                nc.scalar.copy(
                    outs[0:1, nloc + o0:nloc + o0 + rhalf], pr[:, 1, :])
            nc.sync.dma_start(out_d.rearrange("a b -> (a b)")[None, :], outs[:])

    nc.compile()
    return nc


def kernel(Xnew, meanw0, meanw, varw0, varw, prior_sc, post_prec, perm):
    uv_shards, shared, qbar = _host_tensors(
        Xnew, meanw0, meanw, varw0, varw, prior_sc, post_prec, perm)
    nc = _build_module(NLOC)
    in_maps = [dict(uv=uv_shards[i], **shared) for i in range(NCORES)]
    res = bass_utils.run_bass_kernel_spmd(
        nc, in_maps, core_ids=list(range(NCORES)))
    outs = [res.results[i]["out"] for i in range(NCORES)]
    f_mean = np.concatenate([o[0] for o in outs]).reshape(N, 1).astype(np.float32)
    f_var = (np.concatenate([o[1] for o in outs]).reshape(N, 1)
             * np.float32(qbar)).astype(np.float32)
    return f_mean, f_var

